# revision 36
# baseline (speedup 1.0000x reference)
"""CrossTransFormer attention kernel for 8x Trainium2 NeuronCores (Bass/Tile).

Problem (per batch b, B=8, C=773, P=4096):
    K = Wk @ Xk + bk            [C, P]
    V = Wv @ Xq + bv            [C, P]
    S[i, j] = sum_c K[c, i] * V[c, j] / sqrt(C)       (i, j over P)
    H = softmax(S, axis=i)
    out[k, j] = sum_i Xk[k, i] * H[i, j]              [C, P]

Sharding: data-parallel over batch, one batch per NeuronCore, no collectives.

Algebraic restructure (saves one full projection + all weight transposes):
    S = Xk^T (Wk^T Wv) Xq + u 1^T + 1 w^T   with u = Xk^T (Wk^T bv);
    the j-indexed w term is constant along the softmax axis i and cancels
    exactly -> dropped.
  GT = Wv^T Wk is computed on the PE with both weights in NATURAL layout,
  w1 = Wk^T bv rides along; both fold into the A-projection
  A = G Xq + w1 1^T (w1/ones appended as a K=6 ragged contraction tile).
  The A-proj lhsT tiles are zero-padded to c1=896 so every projection
  chain emits full-128-partition PSUM tiles: the staged A is then
  zero-filled in its ragged rows FOR FREE, letting every S matmul run
  K=128 (a K=5 matmul would take the 32-row-group LDWEIGHTS path which
  cannot background-load under an in-flight full-array matmul).

Fused phase D (per j-block of 512), everything SBUF-resident:
  A-proj: 7 chains of 7 MMs -> ast[128, 7, 512] fp16 (no DRAM staging).
  S-phase: 32 i-tiles, 7-MM chains into triple-buffered PSUM, ACT exp
  (scale=1/sqrt(C)) into es[128, 32, 512] fp16.
  out-phase: 7 k-tile chains of 32 accumulating MMs; the ragged chain
  (5 data rows + softmax-sum row from a ones-column in QT) runs FIRST so
  the reciprocal + partition-broadcast overlap the remaining chains;
  each chain is normalized (DVE) and DMA'd out as it finishes.
"""

import sys

sys.path.insert(0, "/opt/trn_rl_repo")

import numpy as np

import concourse.bacc as bacc
import concourse.mybir as mybir
import concourse.tile as tile
from concourse.bass_utils import run_bass_kernel_spmd
from concourse.masks import make_identity

F32 = mybir.dt.float32
F16 = mybir.dt.float16

C = 773
PT = 128
CT = 7  # ceil(773 / 128) chunks of the channel dim
LC = C - (CT - 1) * PT  # 5 rows in the last chunk
JB = 512  # j-block width (one PSUM bank of fp32)
CW = CT * PT  # c1 padded to 896 for the zero-padded A-proj lhsT


def build(P=4096, n_cores=8):
    NJ = P // JB
    IT = P // PT
    SCALE = float(1.0 / np.sqrt(C))

    nc = bacc.Bacc("TRN2", target_bir_lowering=False, debug=False,
                   num_devices=n_cores)
    Xq = nc.dram_tensor("Xq", [C, P], F32, kind="ExternalInput")
    Xk = nc.dram_tensor("Xk", [C, P], F32, kind="ExternalInput")
    Wk = nc.dram_tensor("Wk", [C, C], F32, kind="ExternalInput")
    bk = nc.dram_tensor("bk", [C], F32, kind="ExternalInput")
    Wv = nc.dram_tensor("Wv", [C, C], F32, kind="ExternalInput")
    bv = nc.dram_tensor("bv", [C], F32, kind="ExternalInput")
    out = nc.dram_tensor("out", [C, P], F32, kind="ExternalOutput")
    del bk  # only enters via a softmax-invariant per-j term

    with tile.TileContext(nc) as tc:
        with tc.tile_pool(name="persist", bufs=1) as persist:
            ident = persist.tile([PT, PT], F16)
            make_identity(nc, ident)

            # Xk fp16 resident, natural [c, p] layout: lhsT tiles for S
            xk16 = persist.tile([PT, CT, P], F16)
            # QT = Xk^T [i-in-tile, it, k]; cols 773..799 zero, col 800
            # all-ones so the ragged out-chain lands softmax sums on PSUM
            # partition 32 (compute-engine partition base must be 0/32/64/96)
            qt = persist.tile([PT, IT, C + 28], F16)
            # exp(S) for one j-block, [i-in-tile, it, j]
            es = persist.tile([PT, IT, JB], F16)
            # GT = Wv^T Wk [c2-part, ct2, c1] fp16, c1 zero-padded to 896
            g16 = persist.tile([PT, CT, CW], F16)
            # packed ragged lhsT: rows 0..4 = GT c2-ragged rows, row 5 = w1
            g6 = persist.tile([8, CW], F16)
            w1row = persist.tile([1, CW], F16)
            ones16 = persist.tile([1, JB], F16)
            nc.gpsimd.memset(ones16[:, :], 1.0)

            # PE warmup: dummy matmuls so the HAM clock-gate opens
            # (4/8 -> 8/8) while the first DMAs are in flight, and the
            # exp activation table loads before the main loop. The warm
            # memset goes FIRST on gpsimd so the warmup matmuls are not
            # queued behind the big persistent-tile memsets below.
            wsb = tc.alloc_tile_pool(name="wsb", bufs=1)
            warm = wsb.tile([PT, JB], F16)
            nc.gpsimd.memset(warm[:, :], 0.0)
            with tc.tile_pool(name="pswarm", bufs=4, space="PSUM") as pswarm:
                for i in range(52):
                    wps = pswarm.tile([PT, JB], F32, tag="wps",
                                      name=f"wps{i}")
                    nc.tensor.matmul(wps[:, :], warm[:, :PT], warm[:, :],
                                     start=True, stop=True,
                                     skip_group_check=True)
                wexp = wsb.tile([1, 16], F32)
                nc.scalar.activation(wexp[:], wps[:1, :16],
                                     mybir.ActivationFunctionType.Exp,
                                     scale=1.0)
            wsb.release()

            nc.gpsimd.memset(qt[:, :, C:], 0.0)
            nc.gpsimd.memset(qt[:, :, C + 27:], 1.0)
            # zero-pad: xk16 ragged c-tile rows 5..127 stay zero; g16/g6/
            # w1row cols 773..895 stay zero (gives all-zero rows 5..127 in
            # every ot=6 A-proj psum -> K=128 S-matmuls read clean zeros)
            nc.gpsimd.memset(xk16[:, CT - 1, :], 0.0)
            nc.gpsimd.memset(g16[:, :, :], 0.0)
            nc.gpsimd.memset(g6[:, :], 0.0)
            nc.gpsimd.memset(w1row[:, :], 0.0)

            # ---- Phase G: GT = Wv^T Wk and w1 = Wk^T bv on the PE ----
            with (
                tc.tile_pool(name="wload", bufs=1) as wload,
                tc.tile_pool(name="psg", bufs=4, space="PSUM") as psg,
            ):
                wk16 = wload.tile([PT, CT, C], F16, tag="wk16")
                wv16 = wload.tile([PT, CT, C], F16, tag="wv16")
                bvcol = wload.tile([PT, CT], F16, tag="bvcol")
                # batched W loads: 3 dma_starts per weight into an f32
                # staging ring, DVE-cast to fp16 (a [pc,773]-per-ot split
                # would cost 14 sync-engine descriptor-gen rounds)
                for Wsrc, dst in ((Wk, wk16), (Wv, wv16)):
                    for lo in (0, 3):
                        ws = wload.tile([PT, 3, C], F32, tag="wstage")
                        nc.sync.dma_start(
                            ws[:, :, :],
                            Wsrc[lo * PT:(lo + 3) * PT, :].rearrange(
                                "(ct p) c -> p ct c", p=PT),
                        )
                        nc.vector.tensor_copy(dst[:, lo:lo + 3, :],
                                              ws[:, :, :])
                    wt = wload.tile([8, C], F32, tag="wtail")
                    nc.sync.dma_start(wt[:LC, :], Wsrc[(CT - 1) * PT:C, :])
                    nc.vector.tensor_copy(dst[:LC, CT - 1, :], wt[:LC, :])
                # bv as columns: per-element-descriptor DMA, but on the
                # gpsimd software queue (which may also cast f32->f16) so
                # it cannot stall the hw rings
                nc.gpsimd.dma_start(
                    bvcol[:, :CT - 1],
                    bv[0:(CT - 1) * PT].rearrange("(ct p) -> p ct", p=PT))
                nc.gpsimd.dma_start(bvcol[:LC, CT - 1:CT],
                                    bv[(CT - 1) * PT:C, None])
                # GT tiles: [c2-tile, c1-chunk], contract over o (7 tiles)
                for ct2 in range(CT):
                    pc2 = PT if ct2 < CT - 1 else LC
                    for h, (j0, j1) in enumerate(((0, JB), (JB, C))):
                        ps = psg.tile([PT, JB], F32, tag="psg")
                        for ot in range(CT):
                            po = PT if ot < CT - 1 else LC
                            nc.tensor.matmul(
                                ps[:pc2, :j1 - j0],
                                wv16[:po, ot, ct2 * PT:ct2 * PT + pc2],
                                wk16[:po, ot, j0:j1],
                                start=(ot == 0),
                                stop=(ot == CT - 1),
                            )
                        nc.any.tensor_copy(g16[:pc2, ct2, j0:j1],
                                           ps[:pc2, :j1 - j0])
                # w1 row: lhsT = bv column (M=1), rhs = Wk chunks
                for h, (j0, j1) in enumerate(((0, JB), (JB, C))):
                    ps = psg.tile([1, JB], F32, tag="psw")
                    for ot in range(CT):
                        po = PT if ot < CT - 1 else LC
                        nc.tensor.matmul(
                            ps[:1, :j1 - j0],
                            bvcol[:po, ot:ot + 1],
                            wk16[:po, ot, j0:j1],
                            start=(ot == 0),
                            stop=(ot == CT - 1),
                        )
                    nc.any.tensor_copy(w1row[:1, j0:j1], ps[:1, :j1 - j0])
                nc.vector.tensor_copy(g6[:LC, :C], g16[:LC, CT - 1, :C])
                # partition-5 base is DMA-only territory (compute engines
                # require base 0/32/64/96)
                nc.sync.dma_start(g6[LC:LC + 1, :C], w1row[:1, :C])

            # pools that span phases B and D
            xqp = tc.alloc_tile_pool(name="xqp", bufs=2)
            xfp = tc.alloc_tile_pool(name="xfp", bufs=3)

            def load_xq(jb):
                js = slice(jb * JB, (jb + 1) * JB)
                xq16 = xqp.tile([PT, CT, JB], F16, tag="xq16",
                                name=f"xq16_{jb}")
                for ct in range(CT):
                    pc = PT if ct < CT - 1 else LC
                    xf = xfp.tile([PT, JB], F32, tag="xqf")
                    nc.sync.dma_start(xf[:pc, :],
                                      Xq[ct * PT:ct * PT + pc, js])
                    nc.vector.tensor_copy(xq16[:pc, ct, :], xf[:pc, :])
                # packed ragged rhs: Xq c2-ragged rows + a ones row
                # (pairs with the w1 row in g6 -> A += w1 * 1^T)
                xq6 = xqp.tile([8, JB], F16, tag="xq6", name=f"xq6_{jb}")
                nc.vector.tensor_copy(xq6[:LC, :], xq16[:LC, CT - 1, :])
                nc.sync.dma_start(xq6[LC:LC + 1, :], ones16[:1, :])
                return xq16, xq6

            # ---- Phase B: stream Xk -> resident fp16 + QT transposes ----
            with tc.tile_pool(name="pst", bufs=4, space="PSUM") as pst:
                for jc in range(NJ):
                    js = slice(jc * JB, (jc + 1) * JB)
                    for ct in range(CT):
                        pc = PT if ct < CT - 1 else LC
                        xf = xfp.tile([PT, JB], F32, tag="xkf")
                        nc.sync.dma_start(
                            xf[:pc, :], Xk[ct * PT:ct * PT + pc, js])
                        nc.vector.tensor_copy(xk16[:pc, ct, js], xf[:pc, :])
                    for sub in range(JB // PT):
                        it = jc * (JB // PT) + sub
                        for kt in range(CT):
                            pk = PT if kt < CT - 1 else LC
                            ps = pst.tile([PT, PT], F16, tag="pst")
                            nc.tensor.transpose(
                                ps[:, :pk],
                                xk16[:pk, kt,
                                     jc * JB + sub * PT:
                                     jc * JB + (sub + 1) * PT],
                                ident[:pk, :pk],
                            )
                            nc.any.tensor_copy(
                                qt[:, it, kt * PT:kt * PT + pk], ps[:, :pk]
                            )
                    if jc == 0:
                        xq_next = load_xq(0)

            # ---- Phase D: fused A-projection + attention main loop ----
            with (
                tc.tile_pool(name="astp", bufs=2) as astp,
                tc.tile_pool(name="op", bufs=2) as op,
                tc.tile_pool(name="rp", bufs=1) as rp,
                tc.tile_pool(name="psA", bufs=2, space="PSUM") as psA,
                tc.tile_pool(name="psS", bufs=3, space="PSUM") as psS,
                tc.tile_pool(name="psO", bufs=3, space="PSUM") as psO,
            ):
                for jb in range(NJ):
                    js = slice(jb * JB, (jb + 1) * JB)
                    xq16, xq6 = xq_next
                    if jb < NJ - 1:
                        xq_next = load_xq(jb + 1)

                    # A-proj: A[:, jblock] = G @ Xq + w1 (ragged K=6 MM
                    # carries both the c2 tail and the bias row); the
                    # zero-padded lhsT makes all 128 psum rows valid
                    ast = astp.tile([PT, CT, JB], F16, tag="ast",
                                    name=f"ast{jb}")
                    for ot in range(CT):
                        ps = psA.tile([PT, JB], F32, tag="a",
                                      name=f"a{jb}_{ot}")
                        for ct2 in range(CT - 1):
                            nc.tensor.matmul(
                                ps[:, :],
                                g16[:, ct2, ot * PT:(ot + 1) * PT],
                                xq16[:, ct2, :],
                                start=(ct2 == 0),
                                stop=False,
                                skip_group_check=True,
                            )
                        nc.tensor.matmul(
                            ps[:, :],
                            g6[:LC + 1, ot * PT:(ot + 1) * PT],
                            xq6[:LC + 1, :],
                            start=False,
                            stop=True,
                            skip_group_check=True,
                        )
                        nc.any.tensor_copy(ast[:, ot, :], ps[:, :])

                    # S-phase: 32 chains of 7 K=128 MMs, exp into es
                    for t in range(IT):
                        ts = slice(t * PT, (t + 1) * PT)
                        ps_s = psS.tile([PT, JB], F32, tag="s",
                                        name=f"s{jb}_{t}")
                        for ct in range(CT):
                            nc.tensor.matmul(
                                ps_s[:, :],
                                xk16[:, ct, ts],
                                ast[:, ct, :],
                                start=(ct == 0),
                                stop=(ct == CT - 1),
                                skip_group_check=True,
                            )
                        nc.scalar.activation(
                            es[:, t, :], ps_s[:],
                            mybir.ActivationFunctionType.Exp, scale=SCALE,
                        )

                    # out-phase: ragged chain (data rows 768..772 + sums
                    # row from the ones column) FIRST, then full chains
                    bc = None
                    for kt in (CT - 1,) + tuple(range(CT - 1)):
                        mk = 33 if kt == CT - 1 else PT
                        ps_o = psO.tile([PT, JB], F32, tag="o",
                                        name=f"o{jb}_{kt}")
                        for t in range(IT):
                            nc.tensor.matmul(
                                ps_o[:mk, :],
                                qt[:, t, kt * PT:kt * PT + mk],
                                es[:, t, :],
                                start=(t == 0),
                                stop=(t == IT - 1),
                                skip_group_check=True,
                            )
                        if kt == CT - 1:
                            recip = rp.tile([1, JB], F32, tag="recip")
                            nc.vector.reciprocal(recip[:], ps_o[32:33, :])
                            bc = rp.tile([PT, JB], F32, tag="bc")
                            nc.gpsimd.partition_broadcast(bc[:], recip[:])
                        mo = LC if kt == CT - 1 else PT
                        osb = op.tile([PT, JB], F32, tag="osb",
                                      name=f"osb{jb}_{kt}")
                        nc.vector.tensor_mul(
                            out=osb[:mo, :], in0=ps_o[:mo, :],
                            in1=bc[:mo, :],
                        )
                        nc.sync.dma_start(
                            out[kt * PT:kt * PT + mo, js], osb[:mo, :])

            xfp.release()
            xqp.release()

    nc.compile()
    return nc


_CACHE = {}


def _get_program(P=4096, n_cores=8):
    key = (P, n_cores)
    if key not in _CACHE:
        _CACHE[key] = build(P, n_cores)
    return _CACHE[key]


def _run(inputs, trace=False, **kw):
    nc = _get_program()
    Xq = np.asarray(inputs["Xq"], dtype=np.float32)
    Xk = np.asarray(inputs["Xk"], dtype=np.float32)
    Wk = np.ascontiguousarray(np.asarray(inputs["Wk"], dtype=np.float32))
    bkv = np.ascontiguousarray(np.asarray(inputs["bk"], dtype=np.float32))
    Wv = np.ascontiguousarray(np.asarray(inputs["Wv"], dtype=np.float32))
    bvv = np.ascontiguousarray(np.asarray(inputs["bv"], dtype=np.float32))
    B = Xq.shape[0]
    in_maps = [
        {
            "Xq": np.ascontiguousarray(Xq[b]),
            "Xk": np.ascontiguousarray(Xk[b]),
            "Wk": Wk,
            "bk": bkv,
            "Wv": Wv,
            "bv": bvv,
        }
        for b in range(B)
    ]
    res = run_bass_kernel_spmd(nc, in_maps, list(range(B)), trace=trace, **kw)
    outs = np.stack([res.results[b]["out"] for b in range(B)], axis=0)
    return outs.astype(np.float32), res


def kernel(**inputs):
    outs, _ = _run(inputs)
    return outs
